# revision 13
# baseline (speedup 1.0000x reference)
"""ChameleonAttention TRN2 kernel: tensor-parallel over heads across 8 cores.

Per core (4 heads): QKV projection (fp32r matmuls), per-head LayerNorm +
affine, NEOX RoPE, causal attention with scores in [tk, tq] layout (no
probability transpose needed), row-parallel output projection. Host slices
weights per head-shard and sums the 8 partial outputs.
"""

import numpy as np
from contextlib import ExitStack

import concourse.bass as bass
import concourse.tile as tile
from concourse import bacc
from concourse import mybir
from concourse.bass_utils import run_bass_kernel_spmd

# Problem constants (hardcoded per contract)
HIDDEN = 4096
N_HEADS = 32
HEAD_DIM = 128
T = 2048
THETA = 10000.0
EPS = 1e-5
N_CORES = 8
HPC = 4  # heads per core
SCALE = HEAD_DIM ** -0.5

P = 128          # partitions
KT = HIDDEN // P  # 32 k-tiles over hidden
NT = T // P       # 16 T-tiles
NTQB = 4          # tq blocks of 512
TQB = 512

F32 = mybir.dt.float32
F32R = mybir.dt.float32r
AF = mybir.ActivationFunctionType
ALU = mybir.AluOpType


def _r(ap):
    return ap.bitcast(F32R)


def build_nc():
    nc = bacc.Bacc("TRN2", target_bir_lowering=False, debug=False,
                   num_devices=N_CORES)

    # DRAM I/O (per-core shards supplied by host)
    hT = nc.dram_tensor("hT", [HIDDEN, T], F32, kind="ExternalInput").ap()
    wq = nc.dram_tensor("wq", [HIDDEN, 512], F32, kind="ExternalInput").ap()
    wk = nc.dram_tensor("wk", [HIDDEN, 512], F32, kind="ExternalInput").ap()
    wv = nc.dram_tensor("wv", [HIDDEN, 512], F32, kind="ExternalInput").ap()
    wo = nc.dram_tensor("wo", [512, HIDDEN], F32, kind="ExternalInput").ap()
    lnw = nc.dram_tensor("lnw", [P, 1024], F32, kind="ExternalInput").ap()
    lnb = nc.dram_tensor("lnb", [P, 1024], F32, kind="ExternalInput").ap()
    cos4 = nc.dram_tensor("cos4", [T, 256], F32, kind="ExternalInput").ap()
    sin4 = nc.dram_tensor("sin4", [T, 256], F32, kind="ExternalInput").ap()
    masks = nc.dram_tensor("masks", [512, 512], F32, kind="ExternalInput").ap()
    eye = nc.dram_tensor("eye", [P, P], F32, kind="ExternalInput").ap()
    consts = nc.dram_tensor("consts", [P, 2], F32, kind="ExternalInput").ap()
    onesr = nc.dram_tensor("onesr", [1, P], F32, kind="ExternalInput").ap()
    out = nc.dram_tensor("out", [T, HIDDEN], F32, kind="ExternalOutput").ap()

    with tile.TileContext(nc) as tc:
        with ExitStack() as ctx:
            ep = ctx.enter_context

            # SBUF pools
            big = ep(tc.tile_pool(name="big", bufs=1))       # w-pass slot / attn
            singles = ep(tc.tile_pool(name="singles", bufs=1))
            hp = ep(tc.tile_pool(name="hp", bufs=3))          # hidden stream
            csp = ep(tc.tile_pool(name="csp", bufs=1))        # cos/sin stream
            stp = ep(tc.tile_pool(name="stp", bufs=2))        # LN/rope stage
            tmp = ep(tc.tile_pool(name="tmp", bufs=4))        # rope temps
            statp = ep(tc.tile_pool(name="statp", bufs=4))    # bn stats
            tsp = ep(tc.tile_pool(name="tsp", bufs=2))        # transpose stage
            qkvh = ep(tc.tile_pool(name="qkvh", bufs=2))      # per-head q/k/v
            pp = ep(tc.tile_pool(name="pp", bufs=2))          # probs
            smp = ep(tc.tile_pool(name="smp", bufs=2))        # l/r rows
            arp = ep(tc.tile_pool(name="arp", bufs=1))        # attn raw copy
            wop = ep(tc.tile_pool(name="wop", bufs=1))        # wo chunks
            outp = ep(tc.tile_pool(name="outp", bufs=2))      # output stage

            # PSUM pools
            psA = ep(tc.tile_pool(name="psA", bufs=4, space="PSUM"))  # acc
            psB = ep(tc.tile_pool(name="psB", bufs=2, space="PSUM"))  # s/tp
            psC = ep(tc.tile_pool(name="psC", bufs=2, space="PSUM"))  # l/rb

            # DRAM scratch
            dp = ep(tc.tile_pool(name="dp", bufs=1, space="DRAM"))
            q_d = dp.tile([P, HPC, T], F32R)
            k_d = dp.tile([P, HPC, T], F32R)
            v_d = dp.tile([P, NT, 512], F32R)

            # Constants
            lnw_sb = singles.tile([P, 1024], F32)
            nc.sync.dma_start(out=lnw_sb, in_=lnw)
            lnb_sb = singles.tile([P, 1024], F32)
            nc.sync.dma_start(out=lnb_sb, in_=lnb)
            eye_sb = singles.tile([P, P], F32)
            nc.sync.dma_start(out=eye_sb, in_=eye)
            mask_sb = singles.tile([P, 4, 512], F32)
            for o4 in range(4):
                nc.sync.dma_start(out=mask_sb[:, o4, :],
                                  in_=masks[o4 * P:(o4 + 1) * P, :])
            ones_m = singles.tile([P, 1], F32R)
            nc.sync.dma_start(out=ones_m, in_=consts.bitcast(F32R)[:, 0:1])
            ones_k = singles.tile([1, P], F32)
            nc.sync.dma_start(out=ones_k, in_=onesr)
            eps_sb = singles.tile([P, 1], F32)
            nc.sync.dma_start(out=eps_sb, in_=consts[:, 1:2])

            # ---------------- Phase A: QKV + LN + RoPE + transpose ------
            for pi, (w_dram, dst) in enumerate(
                    [(wq, q_d), (wk, k_d), (wv, v_d)]):
                w_sb = big.tile([P, KT, 512], F32R, tag="big")
                for k in range(KT):
                    nc.sync.dma_start(
                        out=w_sb[:, k, :],
                        in_=w_dram.bitcast(F32R)[k * P:(k + 1) * P, :])
                for tg in range(4):  # T-groups of 512 rows
                    accs = [psA.tile([P, 512], F32, tag="acc",
                                     name=f"acc_{pi}_{tg}_{i}")
                            for i in range(4)]
                    for k in range(KT):
                        h_sb = hp.tile([P, 512], F32R)
                        nc.sync.dma_start(
                            out=h_sb,
                            in_=hT.bitcast(F32R)[k * P:(k + 1) * P,
                                                 tg * 512:(tg + 1) * 512])
                        for tt in range(4):
                            nc.tensor.matmul(
                                accs[tt],
                                lhsT=_r(h_sb[:, tt * P:(tt + 1) * P]),
                                rhs=_r(w_sb[:, k, :]),
                                start=(k == 0), stop=(k == KT - 1))
                    for tt in range(4):
                        ti = tg * 4 + tt
                        x = accs[tt]
                        if pi == 2:  # v: plain copy to DRAM
                            vst = stp.tile([P, 512], F32R, tag="stage")
                            nc.scalar.copy(vst, x)
                            nc.sync.dma_start(out=v_d[:, ti, :], in_=vst)
                            continue
                        # LayerNorm per head into stage
                        stage = stp.tile([P, 4, P], F32, tag="stage")
                        for j in range(4):
                            xs = x[:, j * P:(j + 1) * P]
                            stats = statp.tile([P, 6], F32, tag="st")
                            nc.vector.bn_stats(out=stats, in_=xs)
                            mv = statp.tile([P, 2], F32, tag="mv")
                            nc.vector.bn_aggr(out=mv, in_=stats)
                            nc.scalar.activation(out=mv[:, 1:2],
                                                 in_=mv[:, 1:2],
                                                 func=AF.Sqrt, bias=eps_sb)
                            nc.vector.reciprocal(out=mv[:, 1:2],
                                                 in_=mv[:, 1:2])
                            nc.vector.tensor_scalar(
                                out=stage[:, j, :], in0=xs,
                                scalar1=mv[:, 0:1], scalar2=mv[:, 1:2],
                                op0=ALU.subtract, op1=ALU.mult)
                        st2 = stage.rearrange("p a b -> p (a b)")
                        off = pi * 512
                        nc.vector.tensor_mul(st2, st2,
                                             lnw_sb[:, off:off + 512])
                        nc.vector.tensor_add(st2, st2,
                                             lnb_sb[:, off:off + 512])
                        # RoPE (NEOX): halves of each head
                        cs = csp.tile([P, 4, 64], F32, tag="cs")
                        nc.sync.dma_start(
                            out=cs,
                            in_=cos4[ti * P:(ti + 1) * P, :]
                            .rearrange("p (a b) -> p a b", a=4))
                        sn = csp.tile([P, 4, 64], F32, tag="sn")
                        nc.sync.dma_start(
                            out=sn,
                            in_=sin4[ti * P:(ti + 1) * P, :]
                            .rearrange("p (a b) -> p a b", a=4))
                        y1 = stage[:, :, 0:64]
                        y2 = stage[:, :, 64:128]
                        ta = tmp.tile([P, 4, 64], F32, tag="ta")
                        tb = tmp.tile([P, 4, 64], F32, tag="tb")
                        td = tmp.tile([P, 4, 64], F32, tag="td")
                        nc.vector.tensor_mul(ta, y1, cs)   # y1*cos
                        nc.vector.tensor_mul(tb, y1, sn)   # y1*sin
                        nc.vector.tensor_mul(td, y2, sn)   # y2*sin
                        nc.vector.tensor_sub(y1, ta, td)   # r1
                        nc.vector.tensor_mul(ta, y2, cs)   # y2*cos
                        nc.vector.tensor_add(y2, ta, tb)   # r2
                        # Transpose each head block -> [d, T] in DRAM
                        for j in range(4):
                            tp = psB.tile([P, P], F32, tag="tp")
                            nc.tensor.transpose(tp, stage[:, j, :], eye_sb)
                            tst = tsp.tile([P, P], F32R, tag="ts")
                            nc.scalar.copy(tst, tp)
                            nc.sync.dma_start(
                                out=dst[:, j, ti * P:(ti + 1) * P], in_=tst)

            # ---------------- Phase B: attention per head ----------------
            attn = singles.tile([P, HPC, T], F32R, name="attn")
            for h in range(HPC):
                q_h = qkvh.tile([P, T], F32R, tag="qh")
                k_h = qkvh.tile([P, T], F32R, tag="kh")
                v_h = qkvh.tile([P, NT, P], F32R, tag="vh")
                for c16 in range(NT):
                    sl = slice(c16 * P, (c16 + 1) * P)
                    nc.sync.dma_start(out=q_h[:, sl], in_=q_d[:, h, sl])
                    nc.sync.dma_start(out=k_h[:, sl], in_=k_d[:, h, sl])
                    nc.sync.dma_start(
                        out=v_h[:, c16, :],
                        in_=v_d[:, c16, h * P:(h + 1) * P])
                for tqb in range(NTQB):
                    ntk = 4 * (tqb + 1)
                    av = psA.tile([P, TQB], F32, tag="acc")
                    l_ps = psC.tile([1, TQB], F32, tag="sm")
                    for tk in range(ntk):
                        s_ps = psB.tile([P, TQB], F32, tag="tp")
                        nc.tensor.matmul(
                            s_ps,
                            lhsT=_r(k_h[:, tk * P:(tk + 1) * P]),
                            rhs=_r(q_h[:, tqb * TQB:(tqb + 1) * TQB]),
                            start=True, stop=True)
                        p_sb = pp.tile([P, TQB], F32R, tag="p")
                        nc.scalar.activation(p_sb, s_ps, AF.Exp, scale=SCALE)
                        if tk >= 4 * tqb:
                            nc.vector.tensor_mul(
                                p_sb, p_sb, mask_sb[:, tk - 4 * tqb, :])
                        nc.tensor.matmul(av, lhsT=_r(v_h[:, tk, :]),
                                         rhs=_r(p_sb),
                                         start=(tk == 0),
                                         stop=(tk == ntk - 1))
                        nc.tensor.matmul(l_ps, lhsT=_r(ones_m),
                                         rhs=_r(p_sb),
                                         start=(tk == 0),
                                         stop=(tk == ntk - 1))
                    l_sb = smp.tile([1, TQB], F32, tag="l")
                    nc.vector.tensor_copy(l_sb, l_ps)
                    r_sb = smp.tile([1, TQB], F32, tag="r")
                    nc.vector.reciprocal_approx_fast(out=r_sb, in_=l_sb)
                    rb = psC.tile([P, TQB], F32, tag="sm")
                    nc.tensor.matmul(rb, lhsT=ones_k, rhs=r_sb,
                                     start=True, stop=True)
                    araw = arp.tile([P, TQB], F32, tag="ar")
                    nc.scalar.copy(araw, av)
                    nc.vector.tensor_mul(
                        attn[:, h, tqb * TQB:(tqb + 1) * TQB], araw, rb)

            # ---------------- Phase C: output projection ----------------
            for nb in range(16):
                wo_c = wop.tile([P, HPC, 256], F32R, tag="wo")
                for b4 in range(HPC):
                    nc.sync.dma_start(
                        out=wo_c[:, b4, :],
                        in_=wo.bitcast(F32R)[b4 * P:(b4 + 1) * P,
                                             nb * 256:(nb + 1) * 256])
                for tt in range(NT):
                    op = psA.tile([P, 256], F32, tag="acc")
                    for h2 in range(HPC):
                        nc.tensor.matmul(
                            op,
                            lhsT=_r(attn[:, h2, tt * P:(tt + 1) * P]),
                            rhs=_r(wo_c[:, h2, :]),
                            start=(h2 == 0), stop=(h2 == HPC - 1))
                    ot = outp.tile([P, 256], F32, tag="ot")
                    if tt % 2 == 0:
                        nc.vector.tensor_copy(ot, op)
                    else:
                        nc.scalar.copy(ot, op)
                    nc.sync.dma_start(
                        out=out[tt * P:(tt + 1) * P,
                                nb * 256:(nb + 1) * 256],
                        in_=ot)
    nc.compile()
    return nc


def make_inputs(positions, hidden_states, w_qkv, w_o,
                q_norm_w, q_norm_b, k_norm_w, k_norm_b):
    """Host-side shard prep. Returns per-core input maps."""
    positions = np.asarray(positions)
    hidden_states = np.asarray(hidden_states, dtype=np.float32)
    w_qkv = np.asarray(w_qkv, dtype=np.float32)
    w_o = np.asarray(w_o, dtype=np.float32)
    q_norm_w = np.asarray(q_norm_w, dtype=np.float32)
    q_norm_b = np.asarray(q_norm_b, dtype=np.float32)
    k_norm_w = np.asarray(k_norm_w, dtype=np.float32)
    k_norm_b = np.asarray(k_norm_b, dtype=np.float32)

    hT = np.ascontiguousarray(hidden_states.T)

    half = HEAD_DIM // 2
    inv_freq = (1.0 / (THETA ** (np.arange(0, half, dtype=np.float32)
                                 * 2.0 / HEAD_DIM))).astype(np.float32)
    ang = positions.astype(np.float32)[:, None] * inv_freq[None, :]
    cos4 = np.ascontiguousarray(np.tile(np.cos(ang), (1, 4)),
                                dtype=np.float32)
    sin4 = np.ascontiguousarray(np.tile(np.sin(ang), (1, 4)),
                                dtype=np.float32)

    ii = np.arange(128)[:, None]
    jj = np.arange(512)[None, :]
    masks = np.concatenate(
        [(jj >= o * 128 + ii).astype(np.float32) for o in range(4)], axis=0)
    masks = np.ascontiguousarray(masks)

    eye = np.eye(P, dtype=np.float32)
    consts_np = np.stack([np.ones(P, np.float32),
                          np.full(P, EPS, np.float32)], axis=1)
    consts_np = np.ascontiguousarray(consts_np)
    onesr_np = np.ones((1, P), dtype=np.float32)

    q_size = N_HEADS * HEAD_DIM
    kv_size = N_HEADS * HEAD_DIM

    in_maps = []
    for c in range(N_CORES):
        r0 = c * 512
        wq_c = np.ascontiguousarray(w_qkv[r0:r0 + 512].T)
        wk_c = np.ascontiguousarray(w_qkv[q_size + r0:q_size + r0 + 512].T)
        wv_c = np.ascontiguousarray(
            w_qkv[q_size + kv_size + r0:q_size + kv_size + r0 + 512].T)
        wo_c = np.ascontiguousarray(w_o[:, r0:r0 + 512].T)
        h0 = c * HPC
        lnw_row = np.concatenate([q_norm_w[h0:h0 + HPC].reshape(-1),
                                  k_norm_w[h0:h0 + HPC].reshape(-1)])
        lnb_row = np.concatenate([q_norm_b[h0:h0 + HPC].reshape(-1),
                                  k_norm_b[h0:h0 + HPC].reshape(-1)])
        lnw_c = np.ascontiguousarray(
            np.broadcast_to(lnw_row[None, :], (P, 1024)), dtype=np.float32)
        lnb_c = np.ascontiguousarray(
            np.broadcast_to(lnb_row[None, :], (P, 1024)), dtype=np.float32)
        in_maps.append({
            "hT": hT, "wq": wq_c, "wk": wk_c, "wv": wv_c, "wo": wo_c,
            "lnw": lnw_c, "lnb": lnb_c, "cos4": cos4, "sin4": sin4,
            "masks": masks, "eye": eye,
            "consts": consts_np, "onesr": onesr_np,
        })
    return in_maps


_NC_CACHE = None


def get_nc():
    global _NC_CACHE
    if _NC_CACHE is None:
        _NC_CACHE = build_nc()
    return _NC_CACHE


def kernel(positions, hidden_states, w_qkv, w_o,
           q_norm_w, q_norm_b, k_norm_w, k_norm_b):
    in_maps = make_inputs(positions, hidden_states, w_qkv, w_o,
                          q_norm_w, q_norm_b, k_norm_w, k_norm_b)
    nc = get_nc()
    res = run_bass_kernel_spmd(nc, in_maps, core_ids=list(range(N_CORES)))
    total = res.results[0]["out"].astype(np.float32)
    for c in range(1, N_CORES):
        total = total + res.results[c]["out"]
    return total


# revision 17
# speedup vs baseline: 1.1485x; 1.1485x over previous
"""ChameleonAttention TRN2 kernel: tensor-parallel over heads across 8 cores.

Per core (4 heads): QKV projection (fp32r matmuls), per-head LayerNorm +
affine, NEOX RoPE, causal attention with scores in [tk, tq] layout (no
probability transpose needed), row-parallel output projection. Host slices
weights per head-shard and sums the 8 partial outputs.
"""

import numpy as np
from contextlib import ExitStack

import concourse.bass as bass
import concourse.tile as tile
from concourse import bacc
from concourse import mybir
from concourse.bass_utils import run_bass_kernel_spmd

# Problem constants (hardcoded per contract)
HIDDEN = 4096
N_HEADS = 32
HEAD_DIM = 128
T = 2048
THETA = 10000.0
EPS = 1e-5
N_CORES = 8
HPC = 4  # heads per core
SCALE = HEAD_DIM ** -0.5

P = 128          # partitions
KT = HIDDEN // P  # 32 k-tiles over hidden
NT = T // P       # 16 T-tiles
NTQB = 4          # tq blocks of 512
TQB = 512

F32 = mybir.dt.float32
F32R = mybir.dt.float32r
AF = mybir.ActivationFunctionType
ALU = mybir.AluOpType


def _r(ap):
    return ap.bitcast(F32R)


def build_nc():
    nc = bacc.Bacc("TRN2", target_bir_lowering=False, debug=False,
                   num_devices=N_CORES)

    # DRAM I/O (per-core shards supplied by host)
    hT = nc.dram_tensor("hT", [HIDDEN, T], F32, kind="ExternalInput").ap()
    wq = nc.dram_tensor("wq", [HIDDEN, 512], F32, kind="ExternalInput").ap()
    wk = nc.dram_tensor("wk", [HIDDEN, 512], F32, kind="ExternalInput").ap()
    wv = nc.dram_tensor("wv", [HIDDEN, 512], F32, kind="ExternalInput").ap()
    wo = nc.dram_tensor("wo", [512, HIDDEN], F32, kind="ExternalInput").ap()
    lnw = nc.dram_tensor("lnw", [P, 1024], F32, kind="ExternalInput").ap()
    lnb = nc.dram_tensor("lnb", [P, 1024], F32, kind="ExternalInput").ap()
    cos4 = nc.dram_tensor("cos4", [T, 256], F32, kind="ExternalInput").ap()
    sin4 = nc.dram_tensor("sin4", [T, 256], F32, kind="ExternalInput").ap()
    masks = nc.dram_tensor("masks", [512, 512], F32, kind="ExternalInput").ap()
    eye = nc.dram_tensor("eye", [P, P], F32, kind="ExternalInput").ap()
    consts = nc.dram_tensor("consts", [P, 2], F32, kind="ExternalInput").ap()
    onesr = nc.dram_tensor("onesr", [1, P], F32, kind="ExternalInput").ap()
    out = nc.dram_tensor("out", [T, HIDDEN], F32, kind="ExternalOutput").ap()

    with tile.TileContext(nc) as tc:
        with ExitStack() as ctx:
            ep = ctx.enter_context

            # SBUF pools
            big = ep(tc.tile_pool(name="big", bufs=1))       # w-pass slot / attn
            singles = ep(tc.tile_pool(name="singles", bufs=1))
            hp = ep(tc.tile_pool(name="hp", bufs=3))          # hidden stream
            csp = ep(tc.tile_pool(name="csp", bufs=1))        # cos/sin stream
            stp = ep(tc.tile_pool(name="stp", bufs=2))        # LN/rope stage
            sqp = ep(tc.tile_pool(name="sqp", bufs=1))        # squares
            tmp = ep(tc.tile_pool(name="tmp", bufs=2))        # rope temps
            statp = ep(tc.tile_pool(name="statp", bufs=4))    # bn stats
            tsp = ep(tc.tile_pool(name="tsp", bufs=2))        # transpose stage
            qkvh = ep(tc.tile_pool(name="qkvh", bufs=2))      # per-head q/k/v
            pp = ep(tc.tile_pool(name="pp", bufs=2))          # probs
            smp = ep(tc.tile_pool(name="smp", bufs=1))        # l/r rows
            arp = ep(tc.tile_pool(name="arp", bufs=1))        # attn raw copy
            wop = ep(tc.tile_pool(name="wop", bufs=1))        # wo chunks
            outp = ep(tc.tile_pool(name="outp", bufs=2))      # output stage

            # PSUM pools
            psA = ep(tc.tile_pool(name="psA", bufs=4, space="PSUM"))  # acc
            psB = ep(tc.tile_pool(name="psB", bufs=3, space="PSUM"))  # s/tp/rb
            psC = ep(tc.tile_pool(name="psC", bufs=1, space="PSUM"))  # l

            # DRAM scratch
            dp = ep(tc.tile_pool(name="dp", bufs=1, space="DRAM"))
            q_d = dp.tile([P, HPC, T], F32R)
            k_d = dp.tile([P, HPC, T], F32R)
            v_d = dp.tile([P, NT, 512], F32R)

            # Constants
            lnw_sb = singles.tile([P, 1024], F32)
            nc.sync.dma_start(out=lnw_sb, in_=lnw)
            lnb_sb = singles.tile([P, 1024], F32)
            nc.sync.dma_start(out=lnb_sb, in_=lnb)
            eye_sb = singles.tile([P, P], F32)
            nc.sync.dma_start(out=eye_sb, in_=eye)
            mask_sb = singles.tile([P, 4, 512], F32)
            for o4 in range(4):
                nc.sync.dma_start(out=mask_sb[:, o4, :],
                                  in_=masks[o4 * P:(o4 + 1) * P, :])
            ones_m = singles.tile([P, 1], F32R)
            nc.sync.dma_start(out=ones_m, in_=consts.bitcast(F32R)[:, 0:1])
            ones_k = singles.tile([1, P], F32)
            nc.sync.dma_start(out=ones_k, in_=onesr)
            eps_sb = singles.tile([P, 1], F32)
            nc.sync.dma_start(out=eps_sb, in_=consts[:, 1:2])

            # ---------------- Phase A: QKV + LN + RoPE + transpose ------
            hT3 = hT.bitcast(F32R).rearrange("(k p) t -> p k t", p=P)
            for pi, (w_dram, dst) in enumerate(
                    [(wq, q_d), (wk, k_d), (wv, v_d)]):
                w_a = big.tile([P, 16, 512], F32R, tag="bigA",
                               name=f"w_a{pi}")
                w_b = big.tile([P, 16, 512], F32R, tag="bigB",
                               name=f"w_b{pi}")
                for k in range(KT):
                    wt = w_a if k < 16 else w_b
                    nc.sync.dma_start(
                        out=wt[:, k % 16, :],
                        in_=w_dram.bitcast(F32R)[k * P:(k + 1) * P, :])
                for tg in range(4):  # T-groups of 512 rows
                    accs = [psA.tile([P, 512], F32, tag="acc",
                                     name=f"acc_{pi}_{tg}_{i}")
                            for i in range(4)]
                    for kj in range(KT // 2):
                        h_sb = hp.tile([P, 2, 512], F32R)
                        nc.sync.dma_start(
                            out=h_sb,
                            in_=hT3[:, 2 * kj:2 * kj + 2,
                                    tg * 512:(tg + 1) * 512])
                        for kk in range(2):
                            k = 2 * kj + kk
                            wt = w_a if k < 16 else w_b
                            for tt in range(4):
                                nc.tensor.matmul(
                                    accs[tt],
                                    lhsT=h_sb[:, kk, tt * P:(tt + 1) * P],
                                    rhs=wt[:, k % 16, :],
                                    start=(k == 0), stop=(k == KT - 1))
                    for tt in range(4):
                        ti = tg * 4 + tt
                        x = accs[tt]
                        if pi == 2:  # v: plain copy to DRAM
                            vst = stp.tile([P, 512], F32R, tag="stage")
                            nc.scalar.copy(vst, x)
                            nc.gpsimd.dma_start(out=v_d[:, ti, :], in_=vst)
                            continue
                        # evict PSUM fast, then batched LayerNorm on SBUF
                        raw = stp.tile([P, 4, P], F32, tag="stage")
                        raw2 = raw.rearrange("p a b -> p (a b)")
                        nc.scalar.copy(raw2, x)
                        sq = sqp.tile([P, 4, P], F32, tag="sq")
                        nc.scalar.activation(
                            sq.rearrange("p a b -> p (a b)"), raw2,
                            AF.Square)
                        s1 = statp.tile([P, 4], F32, tag="s1")
                        nc.vector.tensor_reduce(
                            out=s1, in_=raw, axis=mybir.AxisListType.X,
                            op=ALU.add)
                        s2 = statp.tile([P, 4], F32, tag="s2")
                        nc.vector.tensor_reduce(
                            out=s2, in_=sq, axis=mybir.AxisListType.X,
                            op=ALU.add)
                        mean = statp.tile([P, 4], F32, tag="mean")
                        nc.vector.tensor_scalar_mul(
                            out=mean, in0=s1, scalar1=1.0 / HEAD_DIM)
                        msq = statp.tile([P, 4], F32, tag="msq")
                        nc.scalar.activation(msq, mean, AF.Square)
                        var = statp.tile([P, 4], F32, tag="var")
                        nc.vector.scalar_tensor_tensor(
                            out=var, in0=s2, scalar=1.0 / HEAD_DIM,
                            in1=msq, op0=ALU.mult, op1=ALU.subtract)
                        sd = statp.tile([P, 4], F32, tag="sd")
                        nc.scalar.activation(sd, var, AF.Sqrt, bias=eps_sb)
                        rstd = statp.tile([P, 4], F32, tag="rstd")
                        nc.vector.reciprocal(out=rstd, in_=sd)
                        for j in range(4):
                            nc.vector.tensor_scalar(
                                out=raw[:, j, :], in0=raw[:, j, :],
                                scalar1=mean[:, j:j + 1],
                                scalar2=rstd[:, j:j + 1],
                                op0=ALU.subtract, op1=ALU.mult)
                        off = pi * 512
                        nc.vector.tensor_mul(raw2, raw2,
                                             lnw_sb[:, off:off + 512])
                        nc.vector.tensor_add(raw2, raw2,
                                             lnb_sb[:, off:off + 512])
                        # RoPE (NEOX)
                        cs = csp.tile([P, 4, 64], F32, tag="cs")
                        nc.sync.dma_start(
                            out=cs,
                            in_=cos4[ti * P:(ti + 1) * P, :]
                            .rearrange("p (a b) -> p a b", a=4))
                        sn = csp.tile([P, 4, 64], F32, tag="sn")
                        nc.sync.dma_start(
                            out=sn,
                            in_=sin4[ti * P:(ti + 1) * P, :]
                            .rearrange("p (a b) -> p a b", a=4))
                        y1 = raw[:, :, 0:64]
                        y2 = raw[:, :, 64:128]
                        ta = tmp.tile([P, 4, 64], F32, tag="ta")
                        tb = tmp.tile([P, 4, 64], F32, tag="tb")
                        td = tmp.tile([P, 4, 64], F32, tag="td")
                        nc.vector.tensor_mul(ta, y1, cs)   # y1*cos
                        nc.vector.tensor_mul(tb, y1, sn)   # y1*sin
                        nc.vector.tensor_mul(td, y2, sn)   # y2*sin
                        nc.vector.tensor_sub(y1, ta, td)   # r1
                        nc.vector.tensor_mul(ta, y2, cs)   # y2*cos
                        nc.vector.tensor_add(y2, ta, tb)   # r2
                        # Transpose each head block -> [d, T] in DRAM
                        tst4 = tsp.tile([P, 4, P], F32R, tag="ts")
                        for j in range(4):
                            tp = psB.tile([P, P], F32, tag="tp")
                            nc.tensor.transpose(tp, raw[:, j, :], eye_sb)
                            nc.scalar.copy(tst4[:, j, :], tp)
                        nc.gpsimd.dma_start(
                            out=dst[:, :, ti * P:(ti + 1) * P], in_=tst4)

            # ---------------- Phase B: attention per head ----------------
            attn = singles.tile([P, HPC, T], F32R, name="attn")
            for h in range(HPC):
                q_h = qkvh.tile([P, T], F32R, tag="qh")
                k_h = qkvh.tile([P, T], F32R, tag="kh")
                v_h = qkvh.tile([P, NT, P], F32R, tag="vh", bufs=1)
                for c4 in range(4):
                    sl = slice(c4 * TQB, (c4 + 1) * TQB)
                    nc.sync.dma_start(out=q_h[:, sl], in_=q_d[:, h, sl])
                    nc.sync.dma_start(out=k_h[:, sl], in_=k_d[:, h, sl])
                    nc.sync.dma_start(
                        out=v_h[:, 4 * c4:4 * c4 + 4, :],
                        in_=v_d[:, 4 * c4:4 * c4 + 4, h * P:(h + 1) * P])
                for tqb in range(NTQB):
                    ntk = 4 * (tqb + 1)
                    av = psA.tile([P, TQB], F32, tag="acc")
                    l_ps = psC.tile([1, TQB], F32, tag="l")
                    for tk in range(ntk):
                        s_ps = psB.tile([P, TQB], F32, tag="tp")
                        nc.tensor.matmul(
                            s_ps,
                            lhsT=_r(k_h[:, tk * P:(tk + 1) * P]),
                            rhs=_r(q_h[:, tqb * TQB:(tqb + 1) * TQB]),
                            start=True, stop=True)
                        p_sb = pp.tile([P, TQB], F32R, tag="p")
                        nc.scalar.activation(p_sb, s_ps, AF.Exp, scale=SCALE)
                        if tk >= 4 * tqb:
                            nc.vector.tensor_mul(
                                p_sb, p_sb, mask_sb[:, tk - 4 * tqb, :])
                        nc.tensor.matmul(av, lhsT=_r(v_h[:, tk, :]),
                                         rhs=_r(p_sb),
                                         start=(tk == 0),
                                         stop=(tk == ntk - 1))
                        nc.tensor.matmul(l_ps, lhsT=_r(ones_m),
                                         rhs=_r(p_sb),
                                         start=(tk == 0),
                                         stop=(tk == ntk - 1))
                    l_sb = smp.tile([1, TQB], F32, tag="l")
                    nc.vector.tensor_copy(l_sb, l_ps)
                    r_sb = smp.tile([1, TQB], F32, tag="r")
                    nc.vector.reciprocal_approx_fast(out=r_sb, in_=l_sb)
                    rb = psB.tile([P, TQB], F32, tag="tp")
                    nc.tensor.matmul(rb, lhsT=ones_k, rhs=r_sb,
                                     start=True, stop=True)
                    araw = arp.tile([P, TQB], F32, tag="ar")
                    nc.scalar.copy(araw, av)
                    nc.vector.tensor_mul(
                        attn[:, h, tqb * TQB:(tqb + 1) * TQB], araw, rb)

            # ---------------- Phase C: output projection ----------------
            for nb in range(16):
                wo_c = wop.tile([P, HPC, 256], F32R, tag="wo")
                for b4 in range(HPC):
                    nc.sync.dma_start(
                        out=wo_c[:, b4, :],
                        in_=wo.bitcast(F32R)[b4 * P:(b4 + 1) * P,
                                             nb * 256:(nb + 1) * 256])
                for tt in range(NT):
                    op = psA.tile([P, 256], F32, tag="acc")
                    for h2 in range(HPC):
                        nc.tensor.matmul(
                            op,
                            lhsT=_r(attn[:, h2, tt * P:(tt + 1) * P]),
                            rhs=_r(wo_c[:, h2, :]),
                            start=(h2 == 0), stop=(h2 == HPC - 1))
                    ot = outp.tile([P, 256], F32, tag="ot")
                    if tt % 2 == 0:
                        nc.vector.tensor_copy(ot, op)
                    else:
                        nc.scalar.copy(ot, op)
                    nc.gpsimd.dma_start(
                        out=out[tt * P:(tt + 1) * P,
                                nb * 256:(nb + 1) * 256],
                        in_=ot)
    nc.compile()
    return nc


def make_inputs(positions, hidden_states, w_qkv, w_o,
                q_norm_w, q_norm_b, k_norm_w, k_norm_b):
    """Host-side shard prep. Returns per-core input maps."""
    positions = np.asarray(positions)
    hidden_states = np.asarray(hidden_states, dtype=np.float32)
    w_qkv = np.asarray(w_qkv, dtype=np.float32)
    w_o = np.asarray(w_o, dtype=np.float32)
    q_norm_w = np.asarray(q_norm_w, dtype=np.float32)
    q_norm_b = np.asarray(q_norm_b, dtype=np.float32)
    k_norm_w = np.asarray(k_norm_w, dtype=np.float32)
    k_norm_b = np.asarray(k_norm_b, dtype=np.float32)

    hT = np.ascontiguousarray(hidden_states.T)

    half = HEAD_DIM // 2
    inv_freq = (1.0 / (THETA ** (np.arange(0, half, dtype=np.float32)
                                 * 2.0 / HEAD_DIM))).astype(np.float32)
    ang = positions.astype(np.float32)[:, None] * inv_freq[None, :]
    cos4 = np.ascontiguousarray(np.tile(np.cos(ang), (1, 4)),
                                dtype=np.float32)
    sin4 = np.ascontiguousarray(np.tile(np.sin(ang), (1, 4)),
                                dtype=np.float32)

    ii = np.arange(128)[:, None]
    jj = np.arange(512)[None, :]
    masks = np.concatenate(
        [(jj >= o * 128 + ii).astype(np.float32) for o in range(4)], axis=0)
    masks = np.ascontiguousarray(masks)

    eye = np.eye(P, dtype=np.float32)
    consts_np = np.stack([np.ones(P, np.float32),
                          np.full(P, EPS, np.float32)], axis=1)
    consts_np = np.ascontiguousarray(consts_np)
    onesr_np = np.ones((1, P), dtype=np.float32)

    q_size = N_HEADS * HEAD_DIM
    kv_size = N_HEADS * HEAD_DIM

    in_maps = []
    for c in range(N_CORES):
        r0 = c * 512
        wq_c = np.ascontiguousarray(w_qkv[r0:r0 + 512].T)
        wk_c = np.ascontiguousarray(w_qkv[q_size + r0:q_size + r0 + 512].T)
        wv_c = np.ascontiguousarray(
            w_qkv[q_size + kv_size + r0:q_size + kv_size + r0 + 512].T)
        wo_c = np.ascontiguousarray(w_o[:, r0:r0 + 512].T)
        h0 = c * HPC
        lnw_row = np.concatenate([q_norm_w[h0:h0 + HPC].reshape(-1),
                                  k_norm_w[h0:h0 + HPC].reshape(-1)])
        lnb_row = np.concatenate([q_norm_b[h0:h0 + HPC].reshape(-1),
                                  k_norm_b[h0:h0 + HPC].reshape(-1)])
        lnw_c = np.ascontiguousarray(
            np.broadcast_to(lnw_row[None, :], (P, 1024)), dtype=np.float32)
        lnb_c = np.ascontiguousarray(
            np.broadcast_to(lnb_row[None, :], (P, 1024)), dtype=np.float32)
        in_maps.append({
            "hT": hT, "wq": wq_c, "wk": wk_c, "wv": wv_c, "wo": wo_c,
            "lnw": lnw_c, "lnb": lnb_c, "cos4": cos4, "sin4": sin4,
            "masks": masks, "eye": eye,
            "consts": consts_np, "onesr": onesr_np,
        })
    return in_maps


_NC_CACHE = None


def get_nc():
    global _NC_CACHE
    if _NC_CACHE is None:
        _NC_CACHE = build_nc()
    return _NC_CACHE


def kernel(positions, hidden_states, w_qkv, w_o,
           q_norm_w, q_norm_b, k_norm_w, k_norm_b):
    in_maps = make_inputs(positions, hidden_states, w_qkv, w_o,
                          q_norm_w, q_norm_b, k_norm_w, k_norm_b)
    nc = get_nc()
    res = run_bass_kernel_spmd(nc, in_maps, core_ids=list(range(N_CORES)))
    total = res.results[0]["out"].astype(np.float32)
    for c in range(1, N_CORES):
        total = total + res.results[c]["out"]
    return total


# revision 18
# speedup vs baseline: 1.1486x; 1.0001x over previous
"""ChameleonAttention TRN2 kernel: tensor-parallel over heads across 8 cores.

Per core (4 heads): QKV projection (fp32r matmuls), per-head LayerNorm +
affine, NEOX RoPE, causal attention with scores in [tk, tq] layout (no
probability transpose needed), row-parallel output projection. Host slices
weights per head-shard and sums the 8 partial outputs.
"""

import numpy as np
from contextlib import ExitStack

import concourse.bass as bass
import concourse.tile as tile
from concourse import bacc
from concourse import mybir
from concourse.bass_utils import run_bass_kernel_spmd

# Problem constants (hardcoded per contract)
HIDDEN = 4096
N_HEADS = 32
HEAD_DIM = 128
T = 2048
THETA = 10000.0
EPS = 1e-5
N_CORES = 8
HPC = 4  # heads per core
SCALE = HEAD_DIM ** -0.5

P = 128          # partitions
KT = HIDDEN // P  # 32 k-tiles over hidden
NT = T // P       # 16 T-tiles
NTQB = 4          # tq blocks of 512
TQB = 512

F32 = mybir.dt.float32
F32R = mybir.dt.float32r
AF = mybir.ActivationFunctionType
ALU = mybir.AluOpType


def _r(ap):
    return ap.bitcast(F32R)


def build_nc():
    nc = bacc.Bacc("TRN2", target_bir_lowering=False, debug=False,
                   num_devices=N_CORES)

    # DRAM I/O (per-core shards supplied by host)
    hT = nc.dram_tensor("hT", [HIDDEN, T], F32, kind="ExternalInput").ap()
    wq = nc.dram_tensor("wq", [HIDDEN, 512], F32, kind="ExternalInput").ap()
    wk = nc.dram_tensor("wk", [HIDDEN, 512], F32, kind="ExternalInput").ap()
    wv = nc.dram_tensor("wv", [HIDDEN, 512], F32, kind="ExternalInput").ap()
    wo = nc.dram_tensor("wo", [512, HIDDEN], F32, kind="ExternalInput").ap()
    lnw = nc.dram_tensor("lnw", [P, 1024], F32, kind="ExternalInput").ap()
    lnb = nc.dram_tensor("lnb", [P, 1024], F32, kind="ExternalInput").ap()
    cos4 = nc.dram_tensor("cos4", [T, 256], F32, kind="ExternalInput").ap()
    sin4 = nc.dram_tensor("sin4", [T, 256], F32, kind="ExternalInput").ap()
    masks = nc.dram_tensor("masks", [512, 512], F32, kind="ExternalInput").ap()
    eye = nc.dram_tensor("eye", [P, P], F32, kind="ExternalInput").ap()
    consts = nc.dram_tensor("consts", [P, 2], F32, kind="ExternalInput").ap()
    onesr = nc.dram_tensor("onesr", [1, P], F32, kind="ExternalInput").ap()
    out = nc.dram_tensor("out", [T, HIDDEN], F32, kind="ExternalOutput").ap()

    with tile.TileContext(nc) as tc:
        with ExitStack() as ctx:
            ep = ctx.enter_context

            # SBUF pools
            big = ep(tc.tile_pool(name="big", bufs=1))       # w-pass slot / attn
            singles = ep(tc.tile_pool(name="singles", bufs=1))
            hp = ep(tc.tile_pool(name="hp", bufs=2))          # hidden stream
            csp = ep(tc.tile_pool(name="csp", bufs=1))        # cos/sin stream
            stp = ep(tc.tile_pool(name="stp", bufs=2))        # LN/rope stage
            sqp = ep(tc.tile_pool(name="sqp", bufs=1))        # squares
            tmp = ep(tc.tile_pool(name="tmp", bufs=2))        # rope temps
            statp = ep(tc.tile_pool(name="statp", bufs=4))    # bn stats
            tsp = ep(tc.tile_pool(name="tsp", bufs=2))        # transpose stage
            qkvh = ep(tc.tile_pool(name="qkvh", bufs=2))      # per-head q/k/v
            pp = ep(tc.tile_pool(name="pp", bufs=2))          # probs
            smp = ep(tc.tile_pool(name="smp", bufs=1))        # l/r rows
            arp = ep(tc.tile_pool(name="arp", bufs=1))        # attn raw copy
            wop = ep(tc.tile_pool(name="wop", bufs=1))        # wo chunks
            outp = ep(tc.tile_pool(name="outp", bufs=2))      # output stage

            # PSUM pools
            psA = ep(tc.tile_pool(name="psA", bufs=4, space="PSUM"))  # acc
            psB = ep(tc.tile_pool(name="psB", bufs=3, space="PSUM"))  # s/tp/rb
            psC = ep(tc.tile_pool(name="psC", bufs=1, space="PSUM"))  # l

            # DRAM scratch
            dp = ep(tc.tile_pool(name="dp", bufs=1, space="DRAM"))
            q_d = dp.tile([P, HPC, T], F32R)
            k_d = dp.tile([P, HPC, T], F32R)
            v_d = dp.tile([P, NT, 512], F32R)

            # Constants
            lnw_sb = singles.tile([P, 1024], F32)
            nc.sync.dma_start(out=lnw_sb, in_=lnw)
            lnb_sb = singles.tile([P, 1024], F32)
            nc.sync.dma_start(out=lnb_sb, in_=lnb)
            eye_sb = singles.tile([P, P], F32)
            nc.sync.dma_start(out=eye_sb, in_=eye)
            mask_sb = singles.tile([P, 4, 512], F32)
            for o4 in range(4):
                nc.sync.dma_start(out=mask_sb[:, o4, :],
                                  in_=masks[o4 * P:(o4 + 1) * P, :])
            ones_m = singles.tile([P, 1], F32R)
            nc.sync.dma_start(out=ones_m, in_=consts.bitcast(F32R)[:, 0:1])
            ones_k = singles.tile([1, P], F32)
            nc.sync.dma_start(out=ones_k, in_=onesr)
            eps_sb = singles.tile([P, 1], F32)
            nc.sync.dma_start(out=eps_sb, in_=consts[:, 1:2])

            # ---------------- Phase A: QKV + LN + RoPE + transpose ------
            hT3 = hT.bitcast(F32R).rearrange("(k p) t -> p k t", p=P)
            for pi, (w_dram, dst) in enumerate(
                    [(wq, q_d), (wk, k_d), (wv, v_d)]):
                w_a = big.tile([P, 16, 512], F32R, tag="bigA",
                               name=f"w_a{pi}")
                w_b = big.tile([P, 16, 512], F32R, tag="bigB",
                               name=f"w_b{pi}")
                for k in range(KT):
                    wt = w_a if k < 16 else w_b
                    nc.sync.dma_start(
                        out=wt[:, k % 16, :],
                        in_=w_dram.bitcast(F32R)[k * P:(k + 1) * P, :])
                for tg in range(4):  # T-groups of 512 rows
                    accs = [psA.tile([P, 512], F32, tag="acc",
                                     name=f"acc_{pi}_{tg}_{i}")
                            for i in range(4)]
                    for kj in range(KT // 2):
                        h_sb = hp.tile([P, 2, 512], F32R)
                        nc.sync.dma_start(
                            out=h_sb,
                            in_=hT3[:, 2 * kj:2 * kj + 2,
                                    tg * 512:(tg + 1) * 512])
                        for kk in range(2):
                            k = 2 * kj + kk
                            wt = w_a if k < 16 else w_b
                            for tt in range(4):
                                nc.tensor.matmul(
                                    accs[tt],
                                    lhsT=h_sb[:, kk, tt * P:(tt + 1) * P],
                                    rhs=wt[:, k % 16, :],
                                    start=(k == 0), stop=(k == KT - 1))
                    for tt in range(4):
                        ti = tg * 4 + tt
                        x = accs[tt]
                        if pi == 2:  # v: plain copy to DRAM
                            vst = stp.tile([P, 512], F32R, tag="stage")
                            nc.scalar.copy(vst, x)
                            nc.gpsimd.dma_start(out=v_d[:, ti, :], in_=vst)
                            continue
                        # evict PSUM fast, then batched LayerNorm on SBUF
                        raw = stp.tile([P, 4, P], F32, tag="stage")
                        raw2 = raw.rearrange("p a b -> p (a b)")
                        nc.scalar.copy(raw2, x)
                        sq = sqp.tile([P, 4, P], F32, tag="sq")
                        nc.scalar.activation(
                            sq.rearrange("p a b -> p (a b)"), raw2,
                            AF.Square)
                        s1 = statp.tile([P, 4], F32, tag="s1")
                        nc.vector.tensor_reduce(
                            out=s1, in_=raw, axis=mybir.AxisListType.X,
                            op=ALU.add)
                        s2 = statp.tile([P, 4], F32, tag="s2")
                        nc.vector.tensor_reduce(
                            out=s2, in_=sq, axis=mybir.AxisListType.X,
                            op=ALU.add)
                        mean = statp.tile([P, 4], F32, tag="mean")
                        nc.vector.tensor_scalar_mul(
                            out=mean, in0=s1, scalar1=1.0 / HEAD_DIM)
                        msq = statp.tile([P, 4], F32, tag="msq")
                        nc.scalar.activation(msq, mean, AF.Square)
                        var = statp.tile([P, 4], F32, tag="var")
                        nc.vector.scalar_tensor_tensor(
                            out=var, in0=s2, scalar=1.0 / HEAD_DIM,
                            in1=msq, op0=ALU.mult, op1=ALU.subtract)
                        sd = statp.tile([P, 4], F32, tag="sd")
                        nc.scalar.activation(sd, var, AF.Sqrt, bias=eps_sb)
                        rstd = statp.tile([P, 4], F32, tag="rstd")
                        nc.vector.reciprocal(out=rstd, in_=sd)
                        for j in range(4):
                            nc.vector.tensor_scalar(
                                out=raw[:, j, :], in0=raw[:, j, :],
                                scalar1=mean[:, j:j + 1],
                                scalar2=rstd[:, j:j + 1],
                                op0=ALU.subtract, op1=ALU.mult)
                        off = pi * 512
                        nc.vector.tensor_mul(raw2, raw2,
                                             lnw_sb[:, off:off + 512])
                        nc.vector.tensor_add(raw2, raw2,
                                             lnb_sb[:, off:off + 512])
                        # RoPE (NEOX)
                        cs = csp.tile([P, 4, 64], F32, tag="cs")
                        nc.sync.dma_start(
                            out=cs,
                            in_=cos4[ti * P:(ti + 1) * P, :]
                            .rearrange("p (a b) -> p a b", a=4))
                        sn = csp.tile([P, 4, 64], F32, tag="sn")
                        nc.sync.dma_start(
                            out=sn,
                            in_=sin4[ti * P:(ti + 1) * P, :]
                            .rearrange("p (a b) -> p a b", a=4))
                        y1 = raw[:, :, 0:64]
                        y2 = raw[:, :, 64:128]
                        ta = tmp.tile([P, 4, 64], F32, tag="ta")
                        tb = tmp.tile([P, 4, 64], F32, tag="tb")
                        td = tmp.tile([P, 4, 64], F32, tag="td")
                        nc.vector.tensor_mul(ta, y1, cs)   # y1*cos
                        nc.vector.tensor_mul(tb, y1, sn)   # y1*sin
                        nc.vector.tensor_mul(td, y2, sn)   # y2*sin
                        nc.vector.tensor_sub(y1, ta, td)   # r1
                        nc.vector.tensor_mul(ta, y2, cs)   # y2*cos
                        nc.vector.tensor_add(y2, ta, tb)   # r2
                        # Transpose each head block -> [d, T] in DRAM
                        tst4 = tsp.tile([P, 4, P], F32R, tag="ts")
                        for j in range(4):
                            tp = psB.tile([P, P], F32, tag="tp")
                            nc.tensor.transpose(tp, raw[:, j, :], eye_sb)
                            nc.scalar.copy(tst4[:, j, :], tp)
                        nc.gpsimd.dma_start(
                            out=dst[:, :, ti * P:(ti + 1) * P], in_=tst4)

            # ---------------- Phase B: attention per head ----------------
            attn = singles.tile([P, HPC, T], F32R, name="attn")
            for h in range(HPC):
                q_h = qkvh.tile([P, T], F32R, tag="qh")
                k_h = qkvh.tile([P, T], F32R, tag="kh")
                v_h = qkvh.tile([P, NT, P], F32R, tag="vh", bufs=1)
                for c4 in range(4):
                    sl = slice(c4 * TQB, (c4 + 1) * TQB)
                    nc.sync.dma_start(out=q_h[:, sl], in_=q_d[:, h, sl])
                    nc.sync.dma_start(out=k_h[:, sl], in_=k_d[:, h, sl])
                    nc.sync.dma_start(
                        out=v_h[:, 4 * c4:4 * c4 + 4, :],
                        in_=v_d[:, 4 * c4:4 * c4 + 4, h * P:(h + 1) * P])
                for tqb in range(NTQB):
                    ntk = 4 * (tqb + 1)
                    av = psA.tile([P, TQB], F32, tag="acc")
                    l_ps = psC.tile([1, TQB], F32, tag="l")
                    for tk in range(ntk):
                        s_ps = psB.tile([P, TQB], F32, tag="tp")
                        nc.tensor.matmul(
                            s_ps,
                            lhsT=_r(k_h[:, tk * P:(tk + 1) * P]),
                            rhs=_r(q_h[:, tqb * TQB:(tqb + 1) * TQB]),
                            start=True, stop=True)
                        p_sb = pp.tile([P, TQB], F32R, tag="p")
                        nc.scalar.activation(p_sb, s_ps, AF.Exp, scale=SCALE)
                        if tk >= 4 * tqb:
                            nc.vector.tensor_mul(
                                p_sb, p_sb, mask_sb[:, tk - 4 * tqb, :])
                        nc.tensor.matmul(av, lhsT=_r(v_h[:, tk, :]),
                                         rhs=_r(p_sb),
                                         start=(tk == 0),
                                         stop=(tk == ntk - 1))
                        nc.tensor.matmul(l_ps, lhsT=_r(ones_m),
                                         rhs=_r(p_sb),
                                         start=(tk == 0),
                                         stop=(tk == ntk - 1))
                    l_sb = smp.tile([1, TQB], F32, tag="l")
                    nc.vector.tensor_copy(l_sb, l_ps)
                    r_sb = smp.tile([1, TQB], F32, tag="r")
                    nc.vector.reciprocal_approx_fast(out=r_sb, in_=l_sb)
                    rb = psB.tile([P, TQB], F32, tag="tp")
                    nc.tensor.matmul(rb, lhsT=ones_k, rhs=r_sb,
                                     start=True, stop=True)
                    araw = arp.tile([P, TQB], F32, tag="ar")
                    nc.scalar.copy(araw, av)
                    nc.vector.tensor_mul(
                        attn[:, h, tqb * TQB:(tqb + 1) * TQB], araw, rb)

            # ---------------- Phase C: output projection ----------------
            for nb in range(8):
                wo_c = wop.tile([P, HPC, 512], F32R, tag="wo")
                for b4 in range(HPC):
                    nc.sync.dma_start(
                        out=wo_c[:, b4, :],
                        in_=wo.bitcast(F32R)[b4 * P:(b4 + 1) * P,
                                             nb * 512:(nb + 1) * 512])
                for tt in range(NT):
                    op = psA.tile([P, 512], F32, tag="acc")
                    for h2 in range(HPC):
                        nc.tensor.matmul(
                            op,
                            lhsT=_r(attn[:, h2, tt * P:(tt + 1) * P]),
                            rhs=_r(wo_c[:, h2, :]),
                            start=(h2 == 0), stop=(h2 == HPC - 1))
                    ot = outp.tile([P, 512], F32, tag="ot")
                    if tt % 2 == 0:
                        nc.vector.tensor_copy(ot, op)
                    else:
                        nc.scalar.copy(ot, op)
                    nc.gpsimd.dma_start(
                        out=out[tt * P:(tt + 1) * P,
                                nb * 512:(nb + 1) * 512],
                        in_=ot)
    nc.compile()
    return nc


def make_inputs(positions, hidden_states, w_qkv, w_o,
                q_norm_w, q_norm_b, k_norm_w, k_norm_b):
    """Host-side shard prep. Returns per-core input maps."""
    positions = np.asarray(positions)
    hidden_states = np.asarray(hidden_states, dtype=np.float32)
    w_qkv = np.asarray(w_qkv, dtype=np.float32)
    w_o = np.asarray(w_o, dtype=np.float32)
    q_norm_w = np.asarray(q_norm_w, dtype=np.float32)
    q_norm_b = np.asarray(q_norm_b, dtype=np.float32)
    k_norm_w = np.asarray(k_norm_w, dtype=np.float32)
    k_norm_b = np.asarray(k_norm_b, dtype=np.float32)

    hT = np.ascontiguousarray(hidden_states.T)

    half = HEAD_DIM // 2
    inv_freq = (1.0 / (THETA ** (np.arange(0, half, dtype=np.float32)
                                 * 2.0 / HEAD_DIM))).astype(np.float32)
    ang = positions.astype(np.float32)[:, None] * inv_freq[None, :]
    cos4 = np.ascontiguousarray(np.tile(np.cos(ang), (1, 4)),
                                dtype=np.float32)
    sin4 = np.ascontiguousarray(np.tile(np.sin(ang), (1, 4)),
                                dtype=np.float32)

    ii = np.arange(128)[:, None]
    jj = np.arange(512)[None, :]
    masks = np.concatenate(
        [(jj >= o * 128 + ii).astype(np.float32) for o in range(4)], axis=0)
    masks = np.ascontiguousarray(masks)

    eye = np.eye(P, dtype=np.float32)
    consts_np = np.stack([np.ones(P, np.float32),
                          np.full(P, EPS, np.float32)], axis=1)
    consts_np = np.ascontiguousarray(consts_np)
    onesr_np = np.ones((1, P), dtype=np.float32)

    q_size = N_HEADS * HEAD_DIM
    kv_size = N_HEADS * HEAD_DIM

    in_maps = []
    for c in range(N_CORES):
        r0 = c * 512
        wq_c = np.ascontiguousarray(w_qkv[r0:r0 + 512].T)
        wk_c = np.ascontiguousarray(w_qkv[q_size + r0:q_size + r0 + 512].T)
        wv_c = np.ascontiguousarray(
            w_qkv[q_size + kv_size + r0:q_size + kv_size + r0 + 512].T)
        wo_c = np.ascontiguousarray(w_o[:, r0:r0 + 512].T)
        h0 = c * HPC
        lnw_row = np.concatenate([q_norm_w[h0:h0 + HPC].reshape(-1),
                                  k_norm_w[h0:h0 + HPC].reshape(-1)])
        lnb_row = np.concatenate([q_norm_b[h0:h0 + HPC].reshape(-1),
                                  k_norm_b[h0:h0 + HPC].reshape(-1)])
        lnw_c = np.ascontiguousarray(
            np.broadcast_to(lnw_row[None, :], (P, 1024)), dtype=np.float32)
        lnb_c = np.ascontiguousarray(
            np.broadcast_to(lnb_row[None, :], (P, 1024)), dtype=np.float32)
        in_maps.append({
            "hT": hT, "wq": wq_c, "wk": wk_c, "wv": wv_c, "wo": wo_c,
            "lnw": lnw_c, "lnb": lnb_c, "cos4": cos4, "sin4": sin4,
            "masks": masks, "eye": eye,
            "consts": consts_np, "onesr": onesr_np,
        })
    return in_maps


_NC_CACHE = None


def get_nc():
    global _NC_CACHE
    if _NC_CACHE is None:
        _NC_CACHE = build_nc()
    return _NC_CACHE


def kernel(positions, hidden_states, w_qkv, w_o,
           q_norm_w, q_norm_b, k_norm_w, k_norm_b):
    in_maps = make_inputs(positions, hidden_states, w_qkv, w_o,
                          q_norm_w, q_norm_b, k_norm_w, k_norm_b)
    nc = get_nc()
    res = run_bass_kernel_spmd(nc, in_maps, core_ids=list(range(N_CORES)))
    total = res.results[0]["out"].astype(np.float32)
    for c in range(1, N_CORES):
        total = total + res.results[c]["out"]
    return total
